# revision 6
# baseline (speedup 1.0000x reference)
"""Multi-head attention (B=2, S=2048, D=1024, H=16) on 8 Trainium2 cores.

Sharding: core c handles batch b = c//4 and head group g = c%4 (4 heads,
256 of the 1024 QKV output columns). Each core:

  1. Projects q/k in transposed layout qT/kT [dh, s] (lhsT = W.T column
     slice, rhs = x.T), v in natural layout [s, dh] (lhsT = x.T tile,
     rhs = W.T slice). Biases are added via rank-1 matmul accumulation
     (ones ⊗ bias) into the same PSUM group.
  2. Attention per head in transposed layout: logitsT[sk, sq] tile =
     kT_tile.T @ qT (single K=64 matmul), expw = Exp(scale*logits) on ACT
     (scale = 1/sqrt(D) folded into the activation's free affine),
     masked by multiplying with keepT = (~mask).T in bf16 {0,1} — exact,
     since exp(-1e9) underflows to 0 in fp32 so zeroing exp entries is
     identical to the reference's additive -1e9 mask.
  3. PV with a ones-augmented V: out_augT[dh+1, sq] += [v|1].T @ expw —
     row 64 accumulates the softmax denominator for free.
  4. PE-transposes out_augT back to natural [s, dh] in 128-col blocks,
     normalizes rows by 1/rowsum (per-partition scalar), DMAs out.

Matmuls run in bf16 (inputs cast on host), accumulation in fp32 PSUM.
"""

import numpy as np

B, S, D, H = 2, 2048, 1024, 16
HD = D // H  # 64
HEADS_PER_CORE = 4
COLS = HEADS_PER_CORE * HD  # 256
N_CORES = 8
KT = D // 128  # 8 contraction tiles for projections
ST = S // 128  # 16 s tiles
SCALE = 1.0 / np.sqrt(np.float32(D))

_cache = {}


def _build_nc():
    import concourse.bass as bass
    import concourse.mybir as mybir
    import concourse.tile as tile
    from concourse.masks import make_identity

    f32 = mybir.dt.float32
    bf16 = mybir.dt.bfloat16

    nc = bass.Bass(trn_type="TRN2")

    xT = nc.dram_tensor("xT", [D, S], bf16, kind="ExternalInput")
    wq = nc.dram_tensor("wq", [D, COLS], bf16, kind="ExternalInput")
    wk = nc.dram_tensor("wk", [D, COLS], bf16, kind="ExternalInput")
    wv = nc.dram_tensor("wv", [D, COLS], bf16, kind="ExternalInput")
    bq = nc.dram_tensor("bq", [1, 2, 128], bf16, kind="ExternalInput")
    bk = nc.dram_tensor("bk", [1, 2, 128], bf16, kind="ExternalInput")
    bv = nc.dram_tensor("bv", [1, COLS], bf16, kind="ExternalInput")
    keepT = nc.dram_tensor("keepT", [S, S], bf16, kind="ExternalInput")
    o = nc.dram_tensor("o", [S, COLS], f32, kind="ExternalOutput")

    with tile.TileContext(nc) as tc:
        with (
            tc.tile_pool(name="singles", bufs=1) as singles,
            tc.tile_pool(name="persist", bufs=1) as persist,
            tc.tile_pool(name="big_ps", bufs=2, space="PSUM") as big_ps,
            tc.tile_pool(name="pv_ps", bufs=2, space="PSUM") as pv_ps,
            tc.tile_pool(name="expw", bufs=3) as expw_pool,
            tc.tile_pool(name="expw2", bufs=3) as expw2_pool,
            tc.tile_pool(name="tails", bufs=3) as tails,
        ):
            # ---- constants ----
            ones_row = singles.tile([1, 512], bf16)
            nc.vector.memset(ones_row, 1.0)
            ones_col = singles.tile([1, 128], bf16)
            nc.vector.memset(ones_col, 1.0)
            identity = singles.tile([128, 128], f32)
            make_identity(nc, identity)
            bq_sb = singles.tile([1, 2, 128], bf16)
            nc.gpsimd.dma_start(out=bq_sb, in_=bq[:, :, :])
            bk_sb = singles.tile([1, 2, 128], bf16)
            nc.gpsimd.dma_start(out=bk_sb, in_=bk[:, :, :])
            bv_sb = singles.tile([1, COLS], bf16)
            nc.gpsimd.dma_start(out=bv_sb, in_=bv[:, :])

            # ---- bulk inputs ----
            xT_sb = persist.tile([128, KT, S], bf16)
            for kt in range(KT):
                nc.gpsimd.dma_start(
                    out=xT_sb[:, kt, :], in_=xT[kt * 128 : (kt + 1) * 128, :]
                )
            wq_sb = persist.tile([128, KT, COLS], bf16)
            wk_sb = persist.tile([128, KT, COLS], bf16)
            wv_sb = persist.tile([128, KT, COLS], bf16)
            for w_sb, w_dram in ((wq_sb, wq), (wk_sb, wk), (wv_sb, wv)):
                for kt in range(KT):
                    nc.gpsimd.dma_start(
                        out=w_sb[:, kt, :], in_=w_dram[kt * 128 : (kt + 1) * 128, :]
                    )
            keepT_sb = persist.tile([128, ST, S], bf16)
            for i in range(ST):
                nc.gpsimd.dma_start(
                    out=keepT_sb[:, i, :], in_=keepT[i * 128 : (i + 1) * 128, :]
                )

            # ---- QKV projection ----
            # qT/kT: [128 (2 heads of dh), blk, s]; head h lives at
            # partitions (h%2)*64.. of block h//2.
            qT_sb = persist.tile([128, 2, S], bf16)
            kT_sb = persist.tile([128, 2, S], bf16)
            for w_sb, b_sb, dst in ((wq_sb, bq_sb, qT_sb), (wk_sb, bk_sb, kT_sb)):
                for blk in range(2):
                    for jh in range(2):  # s halves of 1024
                        ps = big_ps.tile([128, 1024], f32, tag="big")
                        for nn in range(2):
                            sl = ps[:, nn * 512 : (nn + 1) * 512]
                            nc.tensor.matmul(
                                sl,
                                lhsT=b_sb[:, blk, :],
                                rhs=ones_row[:, :],
                                start=True,
                                stop=False,
                                skip_group_check=True,
                            )
                            for kt in range(KT):
                                nc.tensor.matmul(
                                    sl,
                                    lhsT=w_sb[:, kt, blk * 128 : (blk + 1) * 128],
                                    rhs=xT_sb[
                                        :, kt, jh * 1024 + nn * 512 : jh * 1024 + (nn + 1) * 512
                                    ],
                                    start=False,
                                    stop=(kt == KT - 1),
                                    skip_group_check=True,
                                )
                        nc.scalar.copy(
                            out=dst[:, blk, jh * 1024 : (jh + 1) * 1024], in_=ps
                        )

            # v in natural layout, augmented with a ones column per head:
            # v_aug[p, st, h, 0:64] = v, v_aug[p, st, h, 64] = 1
            v_aug = persist.tile([128, ST, HEADS_PER_CORE, HD + 1], bf16)
            nc.vector.memset(v_aug[:, :, :, HD : HD + 1], 1.0)
            for st in range(ST):
                psv = pv_ps.tile([128, COLS], f32, tag="pv")
                nc.tensor.matmul(
                    psv,
                    lhsT=ones_col[:, :],
                    rhs=bv_sb[:, :],
                    start=True,
                    stop=False,
                    skip_group_check=True,
                )
                for kt in range(KT):
                    nc.tensor.matmul(
                        psv,
                        lhsT=xT_sb[:, kt, st * 128 : (st + 1) * 128],
                        rhs=wv_sb[:, kt, :],
                        start=False,
                        stop=(kt == KT - 1),
                        skip_group_check=True,
                    )
                nc.vector.tensor_copy(
                    out=v_aug[:, st, :, 0:HD],
                    in_=psv.rearrange("p (h d) -> p h d", h=HEADS_PER_CORE),
                )

            # ---- attention ----
            for h in range(HEADS_PER_CORE):
                blk = h // 2
                po = (h % 2) * 64
                for j in range(2):  # sq blocks of 1024
                    pv = pv_ps.tile([HD + 1, 1024], f32, tag="pv")
                    for i in range(ST):  # sk tiles of 128
                        lg = big_ps.tile([128, 1024], f32, tag="big")
                        for nn in range(2):
                            nc.tensor.matmul(
                                lg[:, nn * 512 : (nn + 1) * 512],
                                lhsT=kT_sb[po : po + 64, blk, i * 128 : (i + 1) * 128],
                                rhs=qT_sb[
                                    po : po + 64,
                                    blk,
                                    j * 1024 + nn * 512 : j * 1024 + (nn + 1) * 512,
                                ],
                                start=True,
                                stop=True,
                                skip_group_check=True,
                            )
                        ex = expw_pool.tile([128, 1024], bf16)
                        nc.scalar.activation(
                            out=ex,
                            in_=lg,
                            func=mybir.ActivationFunctionType.Exp,
                            scale=float(SCALE),
                        )
                        ex2 = expw2_pool.tile([128, 1024], bf16)
                        nc.vector.tensor_mul(
                            out=ex2,
                            in0=ex,
                            in1=keepT_sb[:, i, j * 1024 : (j + 1) * 1024],
                        )
                        for nn in range(2):
                            nc.tensor.matmul(
                                pv[:, nn * 512 : (nn + 1) * 512],
                                lhsT=v_aug[:, i, h, :],
                                rhs=ex2[:, nn * 512 : (nn + 1) * 512],
                                start=(i == 0),
                                stop=(i == ST - 1),
                                skip_group_check=True,
                            )
                    # tail: evict, transpose back to [s, dh], normalize, store
                    pv_sb = tails.tile([HD + 1, 1024], f32, tag="pvsb")
                    nc.vector.tensor_copy(out=pv_sb, in_=pv)
                    for c in range(8):
                        tr = pv_ps.tile([128, HD + 1], f32, tag="pv")
                        nc.tensor.transpose(
                            out=tr,
                            in_=pv_sb[:, c * 128 : (c + 1) * 128],
                            identity=identity[0 : HD + 1, 0 : HD + 1],
                        )
                        rc = tails.tile([128, 1], f32, tag="rc")
                        nc.vector.reciprocal(out=rc, in_=tr[:, HD : HD + 1])
                        ob = tails.tile([128, HD], f32, tag="ob")
                        nc.vector.tensor_scalar_mul(
                            out=ob, in0=tr[:, 0:HD], scalar1=rc
                        )
                        nc.gpsimd.dma_start(
                            out=o[
                                j * 1024 + c * 128 : j * 1024 + (c + 1) * 128,
                                h * HD : (h + 1) * HD,
                            ],
                            in_=ob,
                        )

    # Workaround: this container's walrus encodes at most one sync wait per
    # instruction — split multi-wait instructions into single-wait NoOps.
    _split_multiwait(nc)
    return nc


def _split_multiwait(nc, max_waits: int = 1):
    import concourse.mybir as mybir

    for f in nc.m.functions:
        for blk in f.blocks:
            out = []
            changed = False
            for inst in blk.instructions:
                si = inst.sync_info
                if si is not None and len(si.on_wait) > max_waits:
                    waits = list(si.on_wait)
                    extra = waits[: len(waits) - max_waits]
                    keep = waits[len(waits) - max_waits :]
                    for k, w in enumerate(extra):
                        out.append(
                            mybir.InstNoOp(
                                name=f"{inst.name}-wfx{k}",
                                engine=inst.engine,
                                sync_info=mybir.SyncInfo(on_wait=[w], on_update=[]),
                                bass_nofuse=True,
                            )
                        )
                    inst.sync_info = mybir.SyncInfo(
                        on_wait=keep, on_update=list(si.on_update)
                    )
                    changed = True
                out.append(inst)
            if changed:
                blk.instructions = out


def _prep_in_maps(x, mask, Wq, bq, Wk, bk, Wv, bv):
    import ml_dtypes

    bf16 = ml_dtypes.bfloat16
    x = np.asarray(x, np.float32)
    mask = np.asarray(mask, bool)

    xT_b = [np.ascontiguousarray(x[b].T).astype(bf16) for b in range(B)]
    keepT_b = [
        np.ascontiguousarray((~mask[b, 0]).T).astype(bf16) for b in range(B)
    ]
    WqT = np.asarray(Wq, np.float32).T.astype(bf16)
    WkT = np.asarray(Wk, np.float32).T.astype(bf16)
    WvT = np.asarray(Wv, np.float32).T.astype(bf16)
    bq = np.asarray(bq, np.float32).astype(bf16)
    bk = np.asarray(bk, np.float32).astype(bf16)
    bv = np.asarray(bv, np.float32).astype(bf16)

    in_maps = []
    for c in range(N_CORES):
        b, g = divmod(c, 4)
        cols = slice(g * COLS, (g + 1) * COLS)
        in_maps.append(
            {
                "xT": xT_b[b],
                "wq": np.ascontiguousarray(WqT[:, cols]),
                "wk": np.ascontiguousarray(WkT[:, cols]),
                "wv": np.ascontiguousarray(WvT[:, cols]),
                "bq": np.ascontiguousarray(bq[cols].reshape(1, 2, 128)),
                "bk": np.ascontiguousarray(bk[cols].reshape(1, 2, 128)),
                "bv": np.ascontiguousarray(bv[cols].reshape(1, COLS)),
                "keepT": keepT_b[b],
            }
        )
    return in_maps


def kernel(x, mask, Wq, bq, Wk, bk, Wv, bv, _trace=False):
    from concourse.bass_utils import run_bass_kernel_spmd

    if "nc" not in _cache:
        _cache["nc"] = _build_nc()
    nc = _cache["nc"]

    in_maps = _prep_in_maps(x, mask, Wq, bq, Wk, bk, Wv, bv)
    res = run_bass_kernel_spmd(
        nc, in_maps, core_ids=list(range(N_CORES)), trace=_trace
    )
    _cache["last_result"] = res

    out = np.empty((B, S, D), np.float32)
    for c in range(N_CORES):
        b, g = divmod(c, 4)
        out[b, :, g * COLS : (g + 1) * COLS] = res.results[c]["o"]
    return out
